# revision 28
# baseline (speedup 1.0000x reference)
"""Trainium2 Bass kernel for nn_MHA_2688649527670.

Reference computes, per batch b and head h:
    Q = x Wq_h^T, K = x Wk_h^T, V = x Wv_h^T          ([S, D] each)
    Z = softmax_over_d( (Q K^T / sqrt(D)) V )

No softmax between Q K^T and V, so the chain is associative:
    (Q K^T) V = x @ (Wq_h^T Wk_h G Wv_h^T) / sqrt(D),   G = x^T x  ([D, D])

Per-core plan (8 cores = 4 batches x 2 head-groups of 4 heads):
  - inputs staged fp16 (host cast); Wq/Wk/Wv^T/bias-masks packed into ONE
    DRAM tensor (one DMA issue); Wv host-pre-transposed so the weight
    chain runs without on-chip transposes; all matmuls fp16 (1 cyc/row).
  - xT pre-transposed on host (pure layout prep), loaded as one
    contiguous fp16 DMA; x relaid chunk-major on host for full-rate DMA.
  - softmax needs max-subtraction (logits reach +-1200), but the value
    subtracted cancels exactly in the ratio, so fp16 precision suffices:
    -max is reduced on DVE, transposed on the PE, and injected into the
    y PSUM with one K=8 fp16 matmul per chunk against constant
    head-block masks.
  - exp batched over 2 chunks (FD=1024) on ACT, 1/sqrt(D) folded into
    its scale, bf16 out.
  - sums: bf16 TT-fold (2x mode) + tensor_reduce on DVE; reciprocal via
    the fast custom-DVE approx.
  - normalize multiply on GPSIMD (bf16 x fp32-broadcast -> bf16).
  - output written bf16 (tolerance 2e-2; pipeline error ~8e-3), host
    upcasts to fp32.
"""

import numpy as np
import ml_dtypes

import concourse.bass as bass
import concourse.bacc as bacc
import concourse.mybir as mybir
import concourse.tile as tile
from concourse.bass_utils import run_bass_kernel_spmd
from concourse.masks import make_identity

B, S, D, H = 4, 2048, 128, 8
P = 128
HPC = H // 2              # heads per core
NCH = S // P              # 16 s-chunks of 128 rows
NB = NCH // 2             # 8 softmax batches of 2 chunks
N_CORES = 8
SCALE = 1.0 / float(np.sqrt(D))
F32 = mybir.dt.float32
F16 = mybir.dt.float16
BF16 = mybir.dt.bfloat16
WPK = 5 * 512             # packed: wq | wk | wvt | mask0 | mask1

# tuning knob: which batches run the normalize-multiply on GPSIMD
MULT_GP = set(range(NB))

_PROG = None


def _build_program():
    nc = bacc.Bacc("TRN2", target_bir_lowering=False, debug=False,
                   num_devices=N_CORES)

    x_d = nc.dram_tensor("x", [P, NCH * D], F16, kind="ExternalInput")
    xt_d = nc.dram_tensor("xt", [D, S], F16, kind="ExternalInput")
    wpk_d = nc.dram_tensor("wpk", [P, WPK], F16, kind="ExternalInput")
    # chunk-major output, bf16; host reassembles [HPC, S, D]
    out_d = nc.dram_tensor("out", [NCH, P, HPC * D], BF16,
                           kind="ExternalOutput")

    with tile.TileContext(nc) as tc:
        with (
            tc.tile_pool(name="const", bufs=1) as const,
            tc.tile_pool(name="chain", bufs=1) as chain,
            tc.tile_pool(name="e_pool", bufs=4) as e_pool,
            tc.tile_pool(name="o_pool", bufs=4) as o_pool,
            tc.tile_pool(name="s_pool", bufs=4) as s_pool,
            tc.tile_pool(name="ps_y", bufs=3, space="PSUM") as ps_y,
            tc.tile_pool(name="ps_c", bufs=1, space="PSUM") as ps_c,
            tc.tile_pool(name="ps_t", bufs=1, space="PSUM") as ps_t,
        ):
            ident = const.tile([P, P], F16, tag="ident")
            make_identity(nc, ident)

            # ---- loads (all on the sync HWDGE queue, fewest issues;
            #      weights first so p0t runs during the x load) ----
            wpk_sb = const.tile([P, WPK], F16, tag="wpk_sb")
            nc.scalar.dma_start(wpk_sb, wpk_d.ap())

            x_sb = const.tile([P, NCH, D], F16, tag="x_sb")
            x_v = x_d.ap().rearrange("p (n c) -> p n c", n=NCH)
            for q in range(4):
                nc.sync.dma_start(x_sb[:, 4 * q:4 * q + 4, :],
                                  x_v[:, 4 * q:4 * q + 4, :])
            wq_sb = wpk_sb[:, 0:512].rearrange("p (h c) -> p h c", h=HPC)
            wk_sb = wpk_sb[:, 512:1024].rearrange("p (h c) -> p h c", h=HPC)
            wvt_sb = wpk_sb[:, 1024:1536].rearrange("p (h c) -> p h c", h=HPC)
            masks = [wpk_sb[0:2 * HPC, 1536:2048],
                     wpk_sb[0:2 * HPC, 2048:2560]]

            # xT pre-transposed on host -> plain contiguous DMA
            xT_sb = const.tile([P, S], F16, tag="xT_sb")
            nc.scalar.dma_start(xT_sb, xt_d.ap())

            # ACT exp-table preload (after the scalar-queue DMA issues so it
            # doesn't delay them; still far ahead of the first real exp)
            dummy = const.tile([P, 8], F32, tag="dummy")
            nc.vector.memset(dummy, 0.0)
            nc.scalar.activation(dummy, dummy,
                                 mybir.ActivationFunctionType.Exp)
            xT_c = xT_sb[:].rearrange("p (n c) -> p n c", n=NCH)

            # ---- weight chain (all fp16) ----
            p0_ps = ps_c.tile([P, HPC * D], F32, tag="c_ps")
            for h in range(HPC):
                nc.tensor.matmul(p0_ps[:, h * D:(h + 1) * D],
                                 lhsT=wk_sb[:, h, :], rhs=wq_sb[:, h, :])
            p0t = chain.tile([P, HPC, D], F16, tag="p0t")
            nc.scalar.copy(p0t, p0_ps[:].rearrange("p (h c) -> p h c", h=HPC))

            # ---- G = x^T x (fp16 accumulate over 16 chunks) ----
            g_full = ps_c.tile([P, HPC * D], F32, tag="c_ps")
            g_ps = g_full[:, 0:D]
            for i in range(NCH):
                nc.tensor.matmul(g_ps, lhsT=x_sb[:, i, :], rhs=x_sb[:, i, :],
                                 start=(i == 0), stop=(i == NCH - 1))
            g16 = chain.tile([P, D], F16, tag="g16")
            nc.vector.tensor_copy(g16, g_ps)

            ut_ps = ps_c.tile([P, HPC * D], F32, tag="c_ps")
            for h in range(HPC):
                nc.tensor.matmul(ut_ps[:, h * D:(h + 1) * D],
                                 lhsT=g16, rhs=p0t[:, h, :])
            ut = chain.tile([P, HPC, D], F16, tag="ut")
            nc.scalar.copy(ut, ut_ps[:].rearrange("p (h c) -> p h c", h=HPC))

            m_ps = ps_c.tile([P, HPC * D], F32, tag="c_ps")
            for h in range(HPC):
                nc.tensor.matmul(m_ps[:, h * D:(h + 1) * D],
                                 lhsT=ut[:, h, :], rhs=wvt_sb[:, h, :])
            m16 = chain.tile([P, HPC * D], F16, tag="m16")
            nc.scalar.copy(m16, m_ps)

            # ---- per 2-chunk batch: finals + softmax + store,
            #      software-pipelined so DVE's in-order queue never blocks a
            #      ready max_{b+1} behind a waiting sum_b ----
            o_sb = None
            st = [None] * NB       # per-batch (y_ps, e_sb) carried one stage
            for b in range(NB + 1):
                if b == 0:
                    # batch 0 runs as two 1-chunk mini-stages: a much shorter
                    # final->max->bias->exp detour starts the pipeline ~2us
                    # earlier (a lone chunk's mask rows are the plain head
                    # indicators = rows 0..3 of mask0)
                    y_ps = ps_y.tile([P, 2, HPC * D], F32, tag="y_ps")
                    e_sb = e_pool.tile([P, 2, HPC, D], BF16, tag="e_sb")
                    for j in range(2):
                        nc.tensor.matmul(y_ps[:, j, :],
                                         lhsT=xT_c[:, j, :], rhs=m16,
                                         start=True, stop=False,
                                         skip_group_check=True)
                        negmx = s_pool.tile([P, 2 * HPC], F16, tag="negmx")
                        nc.vector.reduce_max(
                            out=negmx[:, 0:HPC],
                            in_=y_ps[:, j, :].rearrange(
                                "p (h d) -> p h d", h=HPC),
                            axis=mybir.AxisListType.X, negate=True)
                        nmt_ps = ps_t.tile([2 * HPC, P], F16, tag="nmt_ps")
                        nc.tensor.transpose(nmt_ps[0:HPC, :],
                                            negmx[:, 0:HPC], ident)
                        nmt16 = s_pool.tile([2 * HPC, P], F16, tag="nmt16")
                        nc.scalar.copy(nmt16[0:HPC, :], nmt_ps[0:HPC, :])
                        nc.tensor.matmul(y_ps[:, j, :],
                                         lhsT=nmt16[0:HPC, :],
                                         rhs=masks[0][0:HPC, :],
                                         start=False, stop=True,
                                         skip_group_check=True)
                        nc.scalar.activation(
                            e_sb[:, j],
                            y_ps[:, j, :].rearrange("p (h d) -> p h d", h=HPC),
                            mybir.ActivationFunctionType.Exp, scale=SCALE)
                    st[0] = e_sb
                elif b < NB:
                    y_ps = ps_y.tile([P, 2, HPC * D], F32, tag="y_ps")
                    for j in range(2):
                        nc.tensor.matmul(y_ps[:, j, :],
                                         lhsT=xT_c[:, 2 * b + j, :], rhs=m16,
                                         start=True, stop=False,
                                         skip_group_check=True)

                    # -max per (row, head) -> transpose on PE (all fp16)
                    negmx = s_pool.tile([P, 2 * HPC], F16, tag="negmx")
                    nc.vector.reduce_max(
                        out=negmx[:].rearrange("p (c h) -> p c h", c=2),
                        in_=y_ps[:].rearrange("p c (h d) -> p c h d", h=HPC),
                        axis=mybir.AxisListType.X, negate=True)
                    nmt_ps = ps_t.tile([2 * HPC, P], F16, tag="nmt_ps")
                    nc.tensor.transpose(nmt_ps, negmx, ident)
                    nmt16 = s_pool.tile([2 * HPC, P], F16, tag="nmt16")
                    nc.scalar.copy(nmt16, nmt_ps)

                    # y -= max via K=8 fp16 matmul per chunk
                    for j in range(2):
                        nc.tensor.matmul(y_ps[:, j, :], lhsT=nmt16,
                                         rhs=masks[j],
                                         start=False, stop=(j == 1),
                                         skip_group_check=True)

                    # e = exp((y - max) / sqrt(D)), bf16
                    e_sb = e_pool.tile([P, 2, HPC, D], BF16, tag="e_sb")
                    nc.scalar.activation(
                        e_sb,
                        y_ps[:].rearrange("p c (h d) -> p c h d", h=HPC),
                        mybir.ActivationFunctionType.Exp, scale=SCALE)
                    st[b] = e_sb

                if b >= 1:
                    p = b - 1
                    e_sb = st[p]
                    sums = s_pool.tile([P, 2 * HPC], F32, tag="sums")
                    nc.vector.reduce_sum(
                        out=sums[:].rearrange("p (c h) -> p c h", c=2),
                        in_=e_sb, axis=mybir.AxisListType.X)
                    rsum = s_pool.tile([P, 2 * HPC], F32, tag="rsum")
                    nc.vector.reciprocal_approx_fast(rsum, sums)

                    o_sb = o_pool.tile([P, 2, HPC, D], BF16, tag="o_sb")
                    rs4 = rsum[:].rearrange("p (c h) -> p c h", c=2)
                    eng = nc.gpsimd if p in MULT_GP else nc.vector
                    if p < NB - 2:
                        rs_b = rs4[:, :, :, None].to_broadcast((P, 2, HPC, D))
                        eng.tensor_tensor(o_sb, e_sb, rs_b,
                                          mybir.AluOpType.mult)
                        nc.sync.dma_start(
                            out_d.ap()[2 * p:2 * p + 2]
                            .rearrange("c p f -> p c f"),
                            o_sb[:].rearrange("p c h d -> p c (h d)"))
                    else:
                        # tail batches: per-chunk mult + store so the first
                        # chunk's store transfer overlaps the second's mult
                        for c in range(2):
                            rs_c = rs4[:, c:c + 1, :, None].to_broadcast(
                                (P, 1, HPC, D))
                            eng.tensor_tensor(o_sb[:, c:c + 1], e_sb[:, c:c + 1],
                                              rs_c, mybir.AluOpType.mult)
                            nc.sync.dma_start(
                                out_d.ap()[2 * p + c:2 * p + c + 1]
                                .rearrange("c p f -> p c f"),
                                o_sb[:, c:c + 1].rearrange("p c h d -> p c (h d)"))

    nc.compile()
    return nc


def _get_program():
    global _PROG
    if _PROG is None:
        _PROG = _build_program()
    return _PROG


def _make_in_maps(x, W_q, W_k, W_v):
    in_maps = []
    for core in range(N_CORES):
        b, hg = core // 2, core % 2
        sl = slice(hg * HPC * D, (hg + 1) * HPC * D)
        wpk = np.zeros((P, WPK), np.float16)
        # wq/wk: [(h p), c] -> [p, (h c)]
        wpk[:, 0:512] = W_q[sl].reshape(HPC, D, D).transpose(1, 0, 2) \
            .reshape(P, HPC * D).astype(np.float16)
        wpk[:, 512:1024] = W_k[sl].reshape(HPC, D, D).transpose(1, 0, 2) \
            .reshape(P, HPC * D).astype(np.float16)
        # wvt[e, (h c)] = Wv_h[c, e]
        wpk[:, 1024:1536] = W_v[sl].reshape(HPC, D, D).transpose(2, 0, 1) \
            .reshape(D, HPC * D).astype(np.float16)
        # bias masks: for chunk j, row (c*4+h)=(j*4+h) carries head-h block
        for j in range(2):
            for h in range(HPC):
                wpk[4 * j + h, 1536 + 512 * j + h * D:
                    1536 + 512 * j + (h + 1) * D] = 1.0
        xb16 = np.ascontiguousarray(x[b]).astype(np.float16)
        # x relaid as [p, (n c)]: row p holds chunk-major slices
        xp = np.ascontiguousarray(
            xb16.reshape(NCH, P, D).transpose(1, 0, 2).reshape(P, NCH * D))
        in_maps.append({
            "x": xp,
            "xt": np.ascontiguousarray(xb16.T),
            "wpk": wpk,
        })
    return in_maps


def run(x, W_q, W_k, W_v, trace=False, **spmd_kwargs):
    """Run on 8 NeuronCores; returns (Z, BassKernelResults)."""
    nc = _get_program()
    in_maps = _make_in_maps(np.asarray(x, np.float32), np.asarray(W_q, np.float32),
                            np.asarray(W_k, np.float32), np.asarray(W_v, np.float32))
    res = run_bass_kernel_spmd(nc, in_maps, core_ids=list(range(N_CORES)),
                               trace=trace, **spmd_kwargs)
    Z = np.empty((B, H, S, D), np.float32)
    for core in range(N_CORES):
        b, hg = core // 2, core % 2
        o = np.asarray(res.results[core]["out"])          # [16, 128, 512] bf16
        o = o.reshape(NCH, P, HPC, D).transpose(2, 0, 1, 3).reshape(HPC, S, D)
        Z[b, hg * HPC:(hg + 1) * HPC] = o.astype(np.float32)
    return Z, res


def kernel(x, W_q, W_k, W_v):
    Z, _ = run(x, W_q, W_k, W_v, trace=False)
    return Z


# revision 30
# speedup vs baseline: 1.0273x; 1.0273x over previous
"""Trainium2 Bass kernel for nn_MHA_2688649527670.

Reference computes, per batch b and head h:
    Q = x Wq_h^T, K = x Wk_h^T, V = x Wv_h^T          ([S, D] each)
    Z = softmax_over_d( (Q K^T / sqrt(D)) V )

No softmax between Q K^T and V, so the chain is associative:
    (Q K^T) V = x @ (Wq_h^T Wk_h G Wv_h^T) / sqrt(D),   G = x^T x  ([D, D])

Per-core plan (8 cores = 4 batches x 2 head-groups of 4 heads):
  - inputs staged fp16 (host cast); Wq/Wk/Wv^T/bias-masks packed into ONE
    DRAM tensor (one DMA issue); Wv host-pre-transposed so the weight
    chain runs without on-chip transposes; all matmuls fp16 (1 cyc/row).
  - xT pre-transposed on host (pure layout prep), loaded as one
    contiguous fp16 DMA; x relaid chunk-major on host for full-rate DMA.
  - softmax needs max-subtraction (logits reach +-1200), but the value
    subtracted cancels exactly in the ratio, so fp16 precision suffices:
    -max is reduced on DVE, transposed on the PE, and injected into the
    y PSUM with one K=8 fp16 matmul per chunk against constant
    head-block masks.
  - exp batched over 2 chunks (FD=1024) on ACT, 1/sqrt(D) folded into
    its scale, bf16 out.
  - sums: bf16 TT-fold (2x mode) + tensor_reduce on DVE; reciprocal via
    the fast custom-DVE approx.
  - normalize multiply on GPSIMD (bf16 x fp32-broadcast -> bf16).
  - output written bf16 (tolerance 2e-2; pipeline error ~8e-3), host
    upcasts to fp32.
"""

import numpy as np
import ml_dtypes

import concourse.bass as bass
import concourse.bacc as bacc
import concourse.mybir as mybir
import concourse.tile as tile
from concourse.bass_utils import run_bass_kernel_spmd
from concourse.masks import make_identity

B, S, D, H = 4, 2048, 128, 8
P = 128
HPC = H // 2              # heads per core
NCH = S // P              # 16 s-chunks of 128 rows
NB = NCH // 2             # 8 softmax batches of 2 chunks
N_CORES = 8
SCALE = 1.0 / float(np.sqrt(D))
F32 = mybir.dt.float32
F16 = mybir.dt.float16
BF16 = mybir.dt.bfloat16
WPK = 5 * 512             # packed: wq | wk | wvt | mask0 | mask1

# tuning knob: which batches run the normalize-multiply on GPSIMD
MULT_GP = set(range(NB))

_PROG = None


def _build_program():
    nc = bacc.Bacc("TRN2", target_bir_lowering=False, debug=False,
                   num_devices=N_CORES)

    x_d = nc.dram_tensor("x", [P, NCH * D], F16, kind="ExternalInput")
    xt_d = nc.dram_tensor("xt", [D, S], F16, kind="ExternalInput")
    wpk_d = nc.dram_tensor("wpk", [P, WPK], F16, kind="ExternalInput")
    # chunk-major output, bf16; host reassembles [HPC, S, D]
    out_d = nc.dram_tensor("out", [NCH, P, HPC * D], BF16,
                           kind="ExternalOutput")

    with tile.TileContext(nc) as tc:
        with (
            tc.tile_pool(name="const", bufs=1) as const,
            tc.tile_pool(name="chain", bufs=1) as chain,
            tc.tile_pool(name="e_pool", bufs=4) as e_pool,
            tc.tile_pool(name="o_pool", bufs=4) as o_pool,
            tc.tile_pool(name="s_pool", bufs=4) as s_pool,
            tc.tile_pool(name="ps_y", bufs=3, space="PSUM") as ps_y,
            tc.tile_pool(name="ps_c", bufs=1, space="PSUM") as ps_c,
            tc.tile_pool(name="ps_t", bufs=1, space="PSUM") as ps_t,
        ):
            ident = const.tile([P, P], F16, tag="ident")
            make_identity(nc, ident)

            # ---- loads (all on the sync HWDGE queue, fewest issues;
            #      weights first so p0t runs during the x load) ----
            wpk_sb = const.tile([P, WPK], F16, tag="wpk_sb")
            nc.scalar.dma_start(wpk_sb, wpk_d.ap())

            x_sb = const.tile([P, NCH, D], F16, tag="x_sb")
            x_v = x_d.ap().rearrange("p (n c) -> p n c", n=NCH)
            for q in range(4):
                nc.sync.dma_start(x_sb[:, 4 * q:4 * q + 4, :],
                                  x_v[:, 4 * q:4 * q + 4, :])
            wq_sb = wpk_sb[:, 0:512].rearrange("p (h c) -> p h c", h=HPC)
            wk_sb = wpk_sb[:, 512:1024].rearrange("p (h c) -> p h c", h=HPC)
            wvt_sb = wpk_sb[:, 1024:1536].rearrange("p (h c) -> p h c", h=HPC)
            masks = [wpk_sb[0:2 * HPC, 1536:2048],
                     wpk_sb[0:2 * HPC, 2048:2560]]

            # xT pre-transposed on host -> plain contiguous DMA
            xT_sb = const.tile([P, S], F16, tag="xT_sb")
            nc.scalar.dma_start(xT_sb, xt_d.ap())

            # ACT exp-table preload (after the scalar-queue DMA issues so it
            # doesn't delay them; still far ahead of the first real exp)
            dummy = const.tile([P, 8], F32, tag="dummy")
            nc.vector.memset(dummy, 0.0)
            nc.scalar.activation(dummy, dummy,
                                 mybir.ActivationFunctionType.Exp)
            xT_c = xT_sb[:].rearrange("p (n c) -> p n c", n=NCH)

            # ---- weight chain (all fp16) ----
            p0_ps = ps_c.tile([P, HPC * D], F32, tag="c_ps")
            for h in range(HPC):
                nc.tensor.matmul(p0_ps[:, h * D:(h + 1) * D],
                                 lhsT=wk_sb[:, h, :], rhs=wq_sb[:, h, :])
            p0t = chain.tile([P, HPC, D], F16, tag="p0t")
            nc.scalar.copy(p0t, p0_ps[:].rearrange("p (h c) -> p h c", h=HPC))

            # ---- G = x^T x (fp16 accumulate over 16 chunks) ----
            g_full = ps_c.tile([P, HPC * D], F32, tag="c_ps")
            g_ps = g_full[:, 0:D]
            for i in range(NCH):
                nc.tensor.matmul(g_ps, lhsT=x_sb[:, i, :], rhs=x_sb[:, i, :],
                                 start=(i == 0), stop=(i == NCH - 1))
            g16 = chain.tile([P, D], F16, tag="g16")
            nc.vector.tensor_copy(g16, g_ps)

            ut_ps = ps_c.tile([P, HPC * D], F32, tag="c_ps")
            for h in range(HPC):
                nc.tensor.matmul(ut_ps[:, h * D:(h + 1) * D],
                                 lhsT=g16, rhs=p0t[:, h, :])
            ut = chain.tile([P, HPC, D], F16, tag="ut")
            nc.scalar.copy(ut, ut_ps[:].rearrange("p (h c) -> p h c", h=HPC))

            m_ps = ps_c.tile([P, HPC * D], F32, tag="c_ps")
            for h in range(HPC):
                nc.tensor.matmul(m_ps[:, h * D:(h + 1) * D],
                                 lhsT=ut[:, h, :], rhs=wvt_sb[:, h, :])
            m16 = chain.tile([P, HPC * D], F16, tag="m16")
            nc.scalar.copy(m16, m_ps)

            # ---- per 2-chunk batch: finals + softmax + store,
            #      software-pipelined so DVE's in-order queue never blocks a
            #      ready max_{b+1} behind a waiting sum_b ----
            o_sb = None
            st = [None] * NB       # per-batch (y_ps, e_sb) carried one stage
            for b in range(NB + 1):
                if b < NB:
                    y_ps = ps_y.tile([P, 2, HPC * D], F32, tag="y_ps")
                    # per-chunk -max interleaved with the finals: chunk 0's
                    # DVE reduce overlaps chunk 1's PE matmul
                    negmx = s_pool.tile([P, 2 * HPC], F16, tag="negmx")
                    for j in range(2):
                        nc.tensor.matmul(y_ps[:, j, :],
                                         lhsT=xT_c[:, 2 * b + j, :], rhs=m16,
                                         start=True, stop=False,
                                         skip_group_check=True)
                        nc.vector.reduce_max(
                            out=negmx[:, j * HPC:(j + 1) * HPC],
                            in_=y_ps[:, j, :].rearrange(
                                "p (h d) -> p h d", h=HPC),
                            axis=mybir.AxisListType.X, negate=True)
                    nmt_ps = ps_t.tile([2 * HPC, P], F16, tag="nmt_ps")
                    nc.tensor.transpose(nmt_ps, negmx, ident)
                    nmt16 = s_pool.tile([2 * HPC, P], F16, tag="nmt16")
                    nc.scalar.copy(nmt16, nmt_ps)

                    # y -= max via K=8 fp16 matmul per chunk
                    for j in range(2):
                        nc.tensor.matmul(y_ps[:, j, :], lhsT=nmt16,
                                         rhs=masks[j],
                                         start=False, stop=(j == 1),
                                         skip_group_check=True)

                    # e = exp((y - max) / sqrt(D)), bf16
                    e_sb = e_pool.tile([P, 2, HPC, D], BF16, tag="e_sb")
                    nc.scalar.activation(
                        e_sb,
                        y_ps[:].rearrange("p c (h d) -> p c h d", h=HPC),
                        mybir.ActivationFunctionType.Exp, scale=SCALE)
                    st[b] = e_sb

                if b >= 1:
                    p = b - 1
                    e_sb = st[p]
                    sums = s_pool.tile([P, 2 * HPC], F32, tag="sums")
                    nc.vector.reduce_sum(
                        out=sums[:].rearrange("p (c h) -> p c h", c=2),
                        in_=e_sb, axis=mybir.AxisListType.X)
                    rsum = s_pool.tile([P, 2 * HPC], F32, tag="rsum")
                    nc.vector.reciprocal_approx_fast(rsum, sums)

                    o_sb = o_pool.tile([P, 2, HPC, D], BF16, tag="o_sb")
                    rs4 = rsum[:].rearrange("p (c h) -> p c h", c=2)
                    eng = nc.gpsimd if p in MULT_GP else nc.vector
                    if p < NB - 2:
                        rs_b = rs4[:, :, :, None].to_broadcast((P, 2, HPC, D))
                        eng.tensor_tensor(o_sb, e_sb, rs_b,
                                          mybir.AluOpType.mult)
                        nc.sync.dma_start(
                            out_d.ap()[2 * p:2 * p + 2]
                            .rearrange("c p f -> p c f"),
                            o_sb[:].rearrange("p c h d -> p c (h d)"))
                    else:
                        # tail batches: per-chunk mult + store so the first
                        # chunk's store transfer overlaps the second's mult
                        for c in range(2):
                            rs_c = rs4[:, c:c + 1, :, None].to_broadcast(
                                (P, 1, HPC, D))
                            eng.tensor_tensor(o_sb[:, c:c + 1], e_sb[:, c:c + 1],
                                              rs_c, mybir.AluOpType.mult)
                            nc.sync.dma_start(
                                out_d.ap()[2 * p + c:2 * p + c + 1]
                                .rearrange("c p f -> p c f"),
                                o_sb[:, c:c + 1].rearrange("p c h d -> p c (h d)"))

    nc.compile()
    return nc


def _get_program():
    global _PROG
    if _PROG is None:
        _PROG = _build_program()
    return _PROG


def _make_in_maps(x, W_q, W_k, W_v):
    in_maps = []
    for core in range(N_CORES):
        b, hg = core // 2, core % 2
        sl = slice(hg * HPC * D, (hg + 1) * HPC * D)
        wpk = np.zeros((P, WPK), np.float16)
        # wq/wk: [(h p), c] -> [p, (h c)]
        wpk[:, 0:512] = W_q[sl].reshape(HPC, D, D).transpose(1, 0, 2) \
            .reshape(P, HPC * D).astype(np.float16)
        wpk[:, 512:1024] = W_k[sl].reshape(HPC, D, D).transpose(1, 0, 2) \
            .reshape(P, HPC * D).astype(np.float16)
        # wvt[e, (h c)] = Wv_h[c, e]
        wpk[:, 1024:1536] = W_v[sl].reshape(HPC, D, D).transpose(2, 0, 1) \
            .reshape(D, HPC * D).astype(np.float16)
        # bias masks: for chunk j, row (c*4+h)=(j*4+h) carries head-h block
        for j in range(2):
            for h in range(HPC):
                wpk[4 * j + h, 1536 + 512 * j + h * D:
                    1536 + 512 * j + (h + 1) * D] = 1.0
        xb16 = np.ascontiguousarray(x[b]).astype(np.float16)
        # x relaid as [p, (n c)]: row p holds chunk-major slices
        xp = np.ascontiguousarray(
            xb16.reshape(NCH, P, D).transpose(1, 0, 2).reshape(P, NCH * D))
        in_maps.append({
            "x": xp,
            "xt": np.ascontiguousarray(xb16.T),
            "wpk": wpk,
        })
    return in_maps


def run(x, W_q, W_k, W_v, trace=False, **spmd_kwargs):
    """Run on 8 NeuronCores; returns (Z, BassKernelResults)."""
    nc = _get_program()
    in_maps = _make_in_maps(np.asarray(x, np.float32), np.asarray(W_q, np.float32),
                            np.asarray(W_k, np.float32), np.asarray(W_v, np.float32))
    res = run_bass_kernel_spmd(nc, in_maps, core_ids=list(range(N_CORES)),
                               trace=trace, **spmd_kwargs)
    Z = np.empty((B, H, S, D), np.float32)
    for core in range(N_CORES):
        b, hg = core // 2, core % 2
        o = np.asarray(res.results[core]["out"])          # [16, 128, 512] bf16
        o = o.reshape(NCH, P, HPC, D).transpose(2, 0, 1, 3).reshape(HPC, S, D)
        Z[b, hg * HPC:(hg + 1) * HPC] = o.astype(np.float32)
    return Z, res


def kernel(x, W_q, W_k, W_v):
    Z, _ = run(x, W_q, W_k, W_v, trace=False)
    return Z


# revision 31
# speedup vs baseline: 1.0551x; 1.0270x over previous
"""Trainium2 Bass kernel for nn_MHA_2688649527670.

Reference computes, per batch b and head h:
    Q = x Wq_h^T, K = x Wk_h^T, V = x Wv_h^T          ([S, D] each)
    Z = softmax_over_d( (Q K^T / sqrt(D)) V )

No softmax between Q K^T and V, so the chain is associative:
    (Q K^T) V = x @ (Wq_h^T Wk_h G Wv_h^T) / sqrt(D),   G = x^T x  ([D, D])

Per-core plan (8 cores = 4 batches x 2 head-groups of 4 heads):
  - inputs staged fp16 (host cast); Wq/Wk/Wv^T/bias-masks packed into ONE
    DRAM tensor (one DMA issue); Wv host-pre-transposed so the weight
    chain runs without on-chip transposes; all matmuls fp16 (1 cyc/row).
  - xT pre-transposed on host (pure layout prep), loaded as one
    contiguous fp16 DMA; x relaid chunk-major on host for full-rate DMA.
  - softmax needs max-subtraction (logits reach +-1200), but the value
    subtracted cancels exactly in the ratio, so fp16 precision suffices:
    -max is reduced on DVE, transposed on the PE, and injected into the
    y PSUM with one K=8 fp16 matmul per chunk against constant
    head-block masks.
  - exp batched over 2 chunks (FD=1024) on ACT, 1/sqrt(D) folded into
    its scale, bf16 out.
  - sums: bf16 TT-fold (2x mode) + tensor_reduce on DVE; reciprocal via
    the fast custom-DVE approx.
  - normalize multiply on GPSIMD (bf16 x fp32-broadcast -> bf16).
  - output written bf16 (tolerance 2e-2; pipeline error ~8e-3), host
    upcasts to fp32.
"""

import numpy as np
import ml_dtypes

import concourse.bass as bass
import concourse.bacc as bacc
import concourse.mybir as mybir
import concourse.tile as tile
from concourse.bass_utils import run_bass_kernel_spmd
from concourse.masks import make_identity

B, S, D, H = 4, 2048, 128, 8
P = 128
HPC = H // 2              # heads per core
NCH = S // P              # 16 s-chunks of 128 rows
NB = NCH // 2             # 8 softmax batches of 2 chunks
N_CORES = 8
SCALE = 1.0 / float(np.sqrt(D))
F32 = mybir.dt.float32
F16 = mybir.dt.float16
BF16 = mybir.dt.bfloat16
WPK = 5 * 512             # packed: wq | wk | wvt | mask0 | mask1

# tuning knob: which batches run the normalize-multiply on GPSIMD
MULT_GP = set(range(NB))

_PROG = None


def _build_program():
    nc = bacc.Bacc("TRN2", target_bir_lowering=False, debug=False,
                   num_devices=N_CORES)

    x_d = nc.dram_tensor("x", [P, NCH * D], F16, kind="ExternalInput")
    xt_d = nc.dram_tensor("xt", [D, S], F16, kind="ExternalInput")
    wpk_d = nc.dram_tensor("wpk", [P, WPK], F16, kind="ExternalInput")
    # chunk-major output, bf16; host reassembles [HPC, S, D]
    out_d = nc.dram_tensor("out", [NCH, P, HPC * D], BF16,
                           kind="ExternalOutput")

    with tile.TileContext(nc) as tc:
        with (
            tc.tile_pool(name="const", bufs=1) as const,
            tc.tile_pool(name="chain", bufs=1) as chain,
            tc.tile_pool(name="e_pool", bufs=4) as e_pool,
            tc.tile_pool(name="o_pool", bufs=4) as o_pool,
            tc.tile_pool(name="s_pool", bufs=4) as s_pool,
            tc.tile_pool(name="ps_y", bufs=3, space="PSUM") as ps_y,
            tc.tile_pool(name="ps_c", bufs=1, space="PSUM") as ps_c,
            tc.tile_pool(name="ps_t", bufs=1, space="PSUM") as ps_t,
        ):
            ident = const.tile([P, P], F16, tag="ident")
            make_identity(nc, ident)

            # ---- loads (all on the sync HWDGE queue, fewest issues;
            #      weights first so p0t runs during the x load) ----
            wpk_sb = const.tile([P, WPK], F16, tag="wpk_sb")
            nc.scalar.dma_start(wpk_sb, wpk_d.ap())

            x_sb = const.tile([P, NCH, D], F16, tag="x_sb")
            x_v = x_d.ap().rearrange("p (n c) -> p n c", n=NCH)
            for q in range(4):
                nc.sync.dma_start(x_sb[:, 4 * q:4 * q + 4, :],
                                  x_v[:, 4 * q:4 * q + 4, :])
            wq_sb = wpk_sb[:, 0:512].rearrange("p (h c) -> p h c", h=HPC)
            wk_sb = wpk_sb[:, 512:1024].rearrange("p (h c) -> p h c", h=HPC)
            wvt_sb = wpk_sb[:, 1024:1536].rearrange("p (h c) -> p h c", h=HPC)
            masks = [wpk_sb[0:2 * HPC, 1536:2048],
                     wpk_sb[0:2 * HPC, 2048:2560]]

            # xT pre-transposed on host -> plain contiguous DMA
            xT_sb = const.tile([P, S], F16, tag="xT_sb")
            nc.scalar.dma_start(xT_sb, xt_d.ap())

            # ACT exp-table preload (after the scalar-queue DMA issues so it
            # doesn't delay them; still far ahead of the first real exp)
            dummy = const.tile([P, 8], F32, tag="dummy")
            nc.vector.memset(dummy, 0.0)
            nc.scalar.activation(dummy, dummy,
                                 mybir.ActivationFunctionType.Exp)
            xT_c = xT_sb[:].rearrange("p (n c) -> p n c", n=NCH)

            # ---- weight chain (all fp16) ----
            p0_ps = ps_c.tile([P, HPC * D], F32, tag="c_ps")
            for h in range(HPC):
                nc.tensor.matmul(p0_ps[:, h * D:(h + 1) * D],
                                 lhsT=wk_sb[:, h, :], rhs=wq_sb[:, h, :])
            p0t = chain.tile([P, HPC, D], F16, tag="p0t")
            nc.scalar.copy(p0t, p0_ps[:].rearrange("p (h c) -> p h c", h=HPC))

            # ---- G = x^T x (fp16 accumulate over 16 chunks) ----
            g_full = ps_c.tile([P, HPC * D], F32, tag="c_ps")
            g_ps = g_full[:, 0:D]
            for i in range(NCH):
                nc.tensor.matmul(g_ps, lhsT=x_sb[:, i, :], rhs=x_sb[:, i, :],
                                 start=(i == 0), stop=(i == NCH - 1))
            g16 = chain.tile([P, D], F16, tag="g16")
            nc.vector.tensor_copy(g16, g_ps)

            ut_ps = ps_c.tile([P, HPC * D], F32, tag="c_ps")
            for h in range(HPC):
                nc.tensor.matmul(ut_ps[:, h * D:(h + 1) * D],
                                 lhsT=g16, rhs=p0t[:, h, :])
            ut = chain.tile([P, HPC, D], F16, tag="ut")
            nc.scalar.copy(ut, ut_ps[:].rearrange("p (h c) -> p h c", h=HPC))

            m_ps = ps_c.tile([P, HPC * D], F32, tag="c_ps")
            for h in range(HPC):
                nc.tensor.matmul(m_ps[:, h * D:(h + 1) * D],
                                 lhsT=ut[:, h, :], rhs=wvt_sb[:, h, :])
            m16 = chain.tile([P, HPC * D], F16, tag="m16")
            nc.scalar.copy(m16, m_ps)

            # ---- per 2-chunk batch: finals + softmax + store,
            #      software-pipelined so DVE's in-order queue never blocks a
            #      ready max_{b+1} behind a waiting sum_b ----
            o_sb = None
            st = [None] * NB       # per-batch (y_ps, e_sb) carried one stage
            for b in range(NB + 1):
                if b < NB:
                    y_ps = ps_y.tile([P, 2, HPC * D], F32, tag="y_ps")
                    for j in range(2):
                        nc.tensor.matmul(y_ps[:, j, :],
                                         lhsT=xT_c[:, 2 * b + j, :], rhs=m16,
                                         start=True, stop=False,
                                         skip_group_check=True)

                    # -max per (row, head) -> transpose on PE (all fp16)
                    negmx = s_pool.tile([P, 2 * HPC], F16, tag="negmx")
                    nc.vector.reduce_max(
                        out=negmx[:].rearrange("p (c h) -> p c h", c=2),
                        in_=y_ps[:].rearrange("p c (h d) -> p c h d", h=HPC),
                        axis=mybir.AxisListType.X, negate=True)
                    nmt_ps = ps_t.tile([2 * HPC, P], F16, tag="nmt_ps")
                    nc.tensor.transpose(nmt_ps, negmx, ident)
                    nmt16 = s_pool.tile([2 * HPC, P], F16, tag="nmt16")
                    nc.scalar.copy(nmt16, nmt_ps)

                    # y -= max via K=8 fp16 matmul per chunk
                    for j in range(2):
                        nc.tensor.matmul(y_ps[:, j, :], lhsT=nmt16,
                                         rhs=masks[j],
                                         start=False, stop=(j == 1),
                                         skip_group_check=True)

                    # e = exp((y - max) / sqrt(D)), bf16
                    e_sb = e_pool.tile([P, 2, HPC, D], BF16, tag="e_sb")
                    nc.scalar.activation(
                        e_sb,
                        y_ps[:].rearrange("p c (h d) -> p c h d", h=HPC),
                        mybir.ActivationFunctionType.Exp, scale=SCALE)
                    st[b] = e_sb

                if b >= 1:
                    p = b - 1
                    e_sb = st[p]
                    sums = s_pool.tile([P, 2 * HPC], F32, tag="sums")
                    nc.vector.reduce_sum(
                        out=sums[:].rearrange("p (c h) -> p c h", c=2),
                        in_=e_sb, axis=mybir.AxisListType.X)
                    rsum = s_pool.tile([P, 2 * HPC], F32, tag="rsum")
                    nc.vector.reciprocal_approx_fast(rsum, sums)

                    o_sb = o_pool.tile([P, 2, HPC, D], BF16, tag="o_sb")
                    rs4 = rsum[:].rearrange("p (c h) -> p c h", c=2)
                    eng = nc.gpsimd if p in MULT_GP else nc.vector
                    if p < NB - 2:
                        rs_b = rs4[:, :, :, None].to_broadcast((P, 2, HPC, D))
                        eng.tensor_tensor(o_sb, e_sb, rs_b,
                                          mybir.AluOpType.mult)
                        nc.sync.dma_start(
                            out_d.ap()[2 * p:2 * p + 2]
                            .rearrange("c p f -> p c f"),
                            o_sb[:].rearrange("p c h d -> p c (h d)"))
                    else:
                        # tail batches: per-chunk mult + store so the first
                        # chunk's store transfer overlaps the second's mult
                        for c in range(2):
                            rs_c = rs4[:, c:c + 1, :, None].to_broadcast(
                                (P, 1, HPC, D))
                            eng.tensor_tensor(o_sb[:, c:c + 1], e_sb[:, c:c + 1],
                                              rs_c, mybir.AluOpType.mult)
                            nc.sync.dma_start(
                                out_d.ap()[2 * p + c:2 * p + c + 1]
                                .rearrange("c p f -> p c f"),
                                o_sb[:, c:c + 1].rearrange("p c h d -> p c (h d)"))

    nc.compile()
    return nc


def _get_program():
    global _PROG
    if _PROG is None:
        _PROG = _build_program()
    return _PROG


def _make_in_maps(x, W_q, W_k, W_v):
    in_maps = []
    for core in range(N_CORES):
        b, hg = core // 2, core % 2
        sl = slice(hg * HPC * D, (hg + 1) * HPC * D)
        wpk = np.zeros((P, WPK), np.float16)
        # wq/wk: [(h p), c] -> [p, (h c)]
        wpk[:, 0:512] = W_q[sl].reshape(HPC, D, D).transpose(1, 0, 2) \
            .reshape(P, HPC * D).astype(np.float16)
        wpk[:, 512:1024] = W_k[sl].reshape(HPC, D, D).transpose(1, 0, 2) \
            .reshape(P, HPC * D).astype(np.float16)
        # wvt[e, (h c)] = Wv_h[c, e]
        wpk[:, 1024:1536] = W_v[sl].reshape(HPC, D, D).transpose(2, 0, 1) \
            .reshape(D, HPC * D).astype(np.float16)
        # bias masks: for chunk j, row (c*4+h)=(j*4+h) carries head-h block
        for j in range(2):
            for h in range(HPC):
                wpk[4 * j + h, 1536 + 512 * j + h * D:
                    1536 + 512 * j + (h + 1) * D] = 1.0
        xb16 = np.ascontiguousarray(x[b]).astype(np.float16)
        # x relaid as [p, (n c)]: row p holds chunk-major slices
        xp = np.ascontiguousarray(
            xb16.reshape(NCH, P, D).transpose(1, 0, 2).reshape(P, NCH * D))
        in_maps.append({
            "x": xp,
            "xt": np.ascontiguousarray(xb16.T),
            "wpk": wpk,
        })
    return in_maps


def run(x, W_q, W_k, W_v, trace=False, **spmd_kwargs):
    """Run on 8 NeuronCores; returns (Z, BassKernelResults)."""
    nc = _get_program()
    in_maps = _make_in_maps(np.asarray(x, np.float32), np.asarray(W_q, np.float32),
                            np.asarray(W_k, np.float32), np.asarray(W_v, np.float32))
    res = run_bass_kernel_spmd(nc, in_maps, core_ids=list(range(N_CORES)),
                               trace=trace, **spmd_kwargs)
    Z = np.empty((B, H, S, D), np.float32)
    for core in range(N_CORES):
        b, hg = core // 2, core % 2
        o = np.asarray(res.results[core]["out"])          # [16, 128, 512] bf16
        o = o.reshape(NCH, P, HPC, D).transpose(2, 0, 1, 3).reshape(HPC, S, D)
        Z[b, hg * HPC:(hg + 1) * HPC] = o.astype(np.float32)
    return Z, res


def kernel(x, W_q, W_k, W_v):
    Z, _ = run(x, W_q, W_k, W_v, trace=False)
    return Z
